# revision 1
# baseline (speedup 1.0000x reference)
"""2D Haar DWT (level 1) Trainium2 Bass kernel — v7 (bf16 middle).

Input  x: [16, 64, 256, 256] f32
Output y: [16, 256, 128, 128] f32, y[n, s*64+c, i, j] = Haar mix s of the
2x2 block x[n, c, 2i:2i+2, 2j:2j+2].

Sharding: pure data parallel over the batch dim — core k gets batches
[2k, 2k+2).

Per-core design (memory-bound, ~67 MB HBM traffic/core):

A group of G=8 channel planes (2 MB, contiguous in DRAM) is loaded as a
pure [128, 4096] f32 reshape: partition p = (c*16 + row//16) holds 16
consecutive rows.  The Haar 0.5 normalization is folded into a ScalarE
pass that also downcasts to bf16 (power-of-2 scale, exact in bf16), so
both butterfly stages become plain adds/subs and stage 1 runs at the
DVE's 2x 16-bit rate:
  scale+cast: ACT  xt = 0.5 * it            (f32 -> bf16)
  stage 1 (vertical):   sum/diff of row pairs   (bf16 -> bf16, 2x DVE)
  stage 2 (horizontal): even +/- odd columns    (bf16 -> f32 out tile)
One 2 MB f32 store per group: DRAM AP [p:128][s:4, stride 4MB][4 KB run]
(s0=add/v0, s1=add/v1, s2=sub/v0, s3=sub/v1).

Loads ride the Sync HWDGE ring, stores the Scalar ring.  Ramp/tail
trimming: group 0's load+scale+stage1 run in two 1 MB halves so compute
starts earlier; the last group's store is split per stage-2 op to halve
the end-of-kernel DMA drain.

Engine busy/core: DMA ~160-175 us (bottleneck), DVE ~107 us, ACT ~70 us.
Numerics: two bf16 roundings -> rel err ~4e-3 (gate is 2e-2).
"""

import sys

sys.path.insert(0, "/opt/trn_rl_repo")

import numpy as np

import concourse.bacc as bacc
import concourse.mybir as mybir
from concourse.tile import TileContext

N_CORES = 8
N_PER_CORE = 2  # batches per core
C = 64  # input channels
H = 256
W = 256
G = 8  # channels per group (2 MB loads, 16 rows/partition)
F32 = mybir.dt.float32
BF16 = mybir.dt.bfloat16


def build_nc():
    nc = bacc.Bacc("TRN2", target_bir_lowering=False, debug=False)
    x = nc.dram_tensor("x", [N_PER_CORE, C, H, W], F32, kind="ExternalInput")
    y = nc.dram_tensor("y", [N_PER_CORE, 4 * C, H // 2, W // 2], F32, kind="ExternalOutput")

    n_groups = N_PER_CORE * C // G

    with TileContext(nc) as tc:
        with (
            tc.tile_pool(name="inpool", bufs=4) as inpool,
            tc.tile_pool(name="xtpool", bufs=3) as xtpool,
            tc.tile_pool(name="sdpool", bufs=3) as sdpool,
            tc.tile_pool(name="outpool", bufs=5) as outpool,
        ):
            for gi in range(n_groups):
                n, c0 = divmod(gi * G, C)

                # --- load: pure reshape of the 2 MB contiguous group.
                # it[p, r, t, w] = x[n, c0 + p//16, 16*(p%16) + 2r + t, w]
                it = inpool.tile([128, G * 512], F32, tag="in")
                itv = it[:].rearrange("p (r t w) -> p r t w", r=G, t=2)
                src = x[n, c0 : c0 + G].rearrange(
                    "c (q r t) w -> (c q) r t w", r=G, t=2
                )
                xt = xtpool.tile([128, G * 512], BF16, tag="xt")
                xtv = xt[:].rearrange("p (r t w) -> p r t w", r=G, t=2)
                sd = sdpool.tile([128, G * 512], BF16, tag="sd")
                sdv = sd[:].rearrange("p (v r w) -> p v r w", v=2, r=G)

                halves = ((0, G // 2), (G // 2, G)) if gi == 0 else ((0, G),)
                for r0, r1 in halves:
                    nc.sync.dma_start(out=itv[:, r0:r1], in_=src[:, r0:r1])
                    # --- Haar 0.5 normalization + downcast to bf16 (exact).
                    nc.scalar.mul(xtv[:, r0:r1], itv[:, r0:r1], 0.5)
                    # --- stage 1 (vertical): rows 2t / 2t+1 in a partition,
                    # 16-bit in/out -> 2x DVE rate.
                    nc.vector.tensor_add(
                        out=sdv[:, 0, r0:r1],
                        in0=xtv[:, r0:r1, 0, :],
                        in1=xtv[:, r0:r1, 1, :],
                    )
                    nc.vector.tensor_sub(
                        out=sdv[:, 1, r0:r1],
                        in0=xtv[:, r0:r1, 0, :],
                        in1=xtv[:, r0:r1, 1, :],
                    )

                # --- stage 2 (horizontal): even/odd column butterfly into
                # one f32 tile holding all four subbands.
                sdj = sd[:].rearrange("p (v r j t) -> p v r j t", v=2, r=G, t=2)
                ot = outpool.tile([128, 4 * G * 128], F32, tag="out")
                ot_v = ot[:].rearrange("p (s r j) -> p s r j", s=4, r=G)
                dst = (
                    y[n]
                    .rearrange("(s c) (q r) j -> s c q (r j)", s=4, r=G)[
                        :, c0 : c0 + G
                    ]
                    .rearrange("s c q f -> (c q) s f")
                )
                ot4 = ot[:].rearrange("p (s f) -> p s f", s=4)
                last = gi == n_groups - 1
                for h, op in ((0, nc.vector.tensor_add), (1, nc.vector.tensor_sub)):
                    op(
                        out=ot_v[:, 2 * h : 2 * h + 2],
                        in0=sdj[..., 0],
                        in1=sdj[..., 1],
                    )
                    if last:
                        # tail trim: stream each subband pair out as soon
                        # as its stage-2 op lands.
                        nc.scalar.dma_start(
                            out=dst[:, 2 * h : 2 * h + 2], in_=ot4[:, 2 * h : 2 * h + 2]
                        )
                if not last:
                    # --- store: one DMA per group.
                    nc.scalar.dma_start(out=dst, in_=ot4)

    nc.finalize()
    return nc


_NC = None


def _get_nc():
    global _NC
    if _NC is None:
        _NC = build_nc()
    return _NC


def kernel(x: np.ndarray) -> np.ndarray:
    from concourse.bass_utils import run_bass_kernel_spmd

    x = np.ascontiguousarray(np.asarray(x), dtype=np.float32)
    assert x.shape == (16, C, H, W), x.shape

    nc = _get_nc()
    in_maps = [
        {"x": x[k * N_PER_CORE : (k + 1) * N_PER_CORE]} for k in range(N_CORES)
    ]
    res = run_bass_kernel_spmd(nc, in_maps, core_ids=list(range(N_CORES)))
    return np.concatenate([r["y"] for r in res.results], axis=0)



# revision 4
# speedup vs baseline: 1.0132x; 1.0132x over previous
"""2D Haar DWT (level 1) Trainium2 Bass kernel — v13 (G=16, deep out pool).

Input  x: [16, 64, 256, 256] f32
Output y: [16, 256, 128, 128] f32, y[n, s*64+c, i, j] = Haar mix s of the
2x2 block x[n, c, 2i:2i+2, 2j:2j+2].

Sharding: pure data parallel over the batch dim — core k gets batches
[2k, 2k+2).

Per-core design (memory-bound, ~67 MB HBM traffic/core, all of which
transits the 16 SDMA engines at ~27 GB/s each — that aggregate is the
roofline; descriptor geometry sets the achieved rate):

A group of G=16 channel planes (4 MB, contiguous in DRAM) is loaded as a
pure [128, 8192] f32 reshape: partition p = (c*8 + row//32) holds 32
consecutive rows, giving 32 KB load descriptors and, on the store side,
16-row/8 KB-contiguous y runs per (partition, subband) — vs 4 KB at
G=8, worth ~1 us of per-engine packet-overhead.  The Haar 0.5
normalization is folded into a ScalarE pass that also downcasts to bf16
(power-of-2 scale, exact in bf16), so both butterfly stages are plain
adds/subs and stage 1 runs at the DVE's 2x 16-bit rate:
  scale+cast: ACT  xt = 0.5 * it            (f32 -> bf16)
  stage 1 (vertical):   sum/diff of row pairs   (bf16 -> bf16)
  stage 2 (horizontal): even +/- odd columns    (bf16 -> f32 out tile)
One 4 MB f32 store per group: DRAM AP [p:128][s:4, stride 4MB][8 KB run].

SBUF/partition budget (208 KB usable): in 32K*2 + xt 16K*1 + sd 16K*1 +
out 32K*3 = 192 KB.  xt/sd single-buffered is free — all their
producers/consumers are ACT/DVE ops that serialize in program order —
while out*3 (12 MB) absorbs the store backlog that builds up because
engine round-robin is packet-granular and load packets are ~3x larger
(stores drain alone for the last ~50 us; harmless as long as stage 2
never blocks on a free out buffer).

Loads ride the Sync HWDGE ring, stores the Scalar ring.  Ramp trim:
group 0's load+scale+stage1 run in four 1 MB pieces and group 1's in
two 2 MB halves so the engines spin up sooner; the last group's store
is split per stage-2 op to trim the end-of-kernel drain.

Measured: 169.5-169.9 us across reps (v7/G=8 reference: 171.2).
Engines ~26.6 GB/s each while busy; ~7 us framework preamble and
~2.5 us completion tail are fixed.  Numerics: two bf16 roundings ->
rel err ~4e-3 (gate is 2e-2).
"""

import sys

sys.path.insert(0, "/opt/trn_rl_repo")

import numpy as np

import concourse.bacc as bacc
import concourse.mybir as mybir
from concourse.tile import TileContext

N_CORES = 8
N_PER_CORE = 2  # batches per core
C = 64  # input channels
H = 256
W = 256
G = 16  # channels per group (4 MB loads, 32 rows/partition)
F32 = mybir.dt.float32
BF16 = mybir.dt.bfloat16


def build_nc():
    nc = bacc.Bacc("TRN2", target_bir_lowering=False, debug=False)
    x = nc.dram_tensor("x", [N_PER_CORE, C, H, W], F32, kind="ExternalInput")
    y = nc.dram_tensor("y", [N_PER_CORE, 4 * C, H // 2, W // 2], F32, kind="ExternalOutput")

    n_groups = N_PER_CORE * C // G
    R = H * G // 128 // 2  # row-pairs per partition (16)

    with TileContext(nc) as tc:
        with (
            tc.tile_pool(name="inpool", bufs=2) as inpool,
            tc.tile_pool(name="xtpool", bufs=1) as xtpool,
            tc.tile_pool(name="sdpool", bufs=1) as sdpool,
            tc.tile_pool(name="outpool", bufs=3) as outpool,
        ):
            for gi in range(n_groups):
                n, c0 = divmod(gi * G, C)

                # --- load: pure reshape of the 4 MB contiguous group.
                # it[p, r, t, w] = x[n, c0 + p//8, 32*(p%8) + 2r + t, w]
                it = inpool.tile([128, R * 512], F32, tag="in")
                itv = it[:].rearrange("p (r t w) -> p r t w", r=R, t=2)
                src = x[n, c0 : c0 + G].rearrange(
                    "c (q r t) w -> (c q) r t w", r=R, t=2
                )
                xt = xtpool.tile([128, R * 512], BF16, tag="xt")
                xtv = xt[:].rearrange("p (r t w) -> p r t w", r=R, t=2)
                sd = sdpool.tile([128, R * 512], BF16, tag="sd")
                sdv = sd[:].rearrange("p (v r w) -> p v r w", v=2, r=R)

                npieces = {0: 4, 1: 2}.get(gi, 1)
                step = R // npieces
                pieces = [(i * step, (i + 1) * step) for i in range(npieces)]
                for r0, r1 in pieces:
                    nc.sync.dma_start(out=itv[:, r0:r1], in_=src[:, r0:r1])
                    # --- Haar 0.5 normalization + downcast to bf16 (exact).
                    nc.scalar.mul(xtv[:, r0:r1], itv[:, r0:r1], 0.5)
                    # --- stage 1 (vertical): rows 2t / 2t+1 in a partition.
                    nc.vector.tensor_add(
                        out=sdv[:, 0, r0:r1],
                        in0=xtv[:, r0:r1, 0, :],
                        in1=xtv[:, r0:r1, 1, :],
                    )
                    nc.vector.tensor_sub(
                        out=sdv[:, 1, r0:r1],
                        in0=xtv[:, r0:r1, 0, :],
                        in1=xtv[:, r0:r1, 1, :],
                    )

                # --- stage 2 (horizontal): even/odd column butterfly into
                # one f32 tile holding all four subbands.
                sdj = sd[:].rearrange("p (v r j t) -> p v r j t", v=2, r=R, t=2)
                ot = outpool.tile([128, 4 * R * 128], F32, tag="out")
                ot_v = ot[:].rearrange("p (s r j) -> p s r j", s=4, r=R)
                dst = (
                    y[n]
                    .rearrange("(s c) (q r) j -> s c q (r j)", s=4, r=R)[
                        :, c0 : c0 + G
                    ]
                    .rearrange("s c q f -> (c q) s f")
                )
                ot4 = ot[:].rearrange("p (s f) -> p s f", s=4)
                last = gi == n_groups - 1
                for h, op in ((0, nc.vector.tensor_add), (1, nc.vector.tensor_sub)):
                    op(
                        out=ot_v[:, 2 * h : 2 * h + 2],
                        in0=sdj[..., 0],
                        in1=sdj[..., 1],
                    )
                    if last:
                        # tail trim: stream each subband pair out as soon
                        # as its stage-2 op lands.
                        nc.scalar.dma_start(
                            out=dst[:, 2 * h : 2 * h + 2], in_=ot4[:, 2 * h : 2 * h + 2]
                        )
                if not last:
                    # --- store: one DMA per group.
                    nc.scalar.dma_start(out=dst, in_=ot4)

    nc.finalize()
    return nc


_NC = None


def _get_nc():
    global _NC
    if _NC is None:
        _NC = build_nc()
    return _NC


def kernel(x: np.ndarray) -> np.ndarray:
    from concourse.bass_utils import run_bass_kernel_spmd

    x = np.ascontiguousarray(np.asarray(x), dtype=np.float32)
    assert x.shape == (16, C, H, W), x.shape

    nc = _get_nc()
    in_maps = [
        {"x": x[k * N_PER_CORE : (k + 1) * N_PER_CORE]} for k in range(N_CORES)
    ]
    res = run_bass_kernel_spmd(nc, in_maps, core_ids=list(range(N_CORES)))
    return np.concatenate([r["y"] for r in res.results], axis=0)


# revision 6
# speedup vs baseline: 1.0170x; 1.0038x over previous
"""2D Haar DWT (level 1) Trainium2 Bass kernel — v13 (G=16, deep out pool).

Input  x: [16, 64, 256, 256] f32
Output y: [16, 256, 128, 128] f32, y[n, s*64+c, i, j] = Haar mix s of the
2x2 block x[n, c, 2i:2i+2, 2j:2j+2].

Sharding: pure data parallel over the batch dim — core k gets batches
[2k, 2k+2).

Per-core design (memory-bound, ~67 MB HBM traffic/core, all of which
transits the 16 SDMA engines at ~27 GB/s each — that aggregate is the
roofline; descriptor geometry sets the achieved rate):

A group of G=16 channel planes (4 MB, contiguous in DRAM) is loaded as a
pure [128, 8192] f32 reshape: partition p = (c*8 + row//32) holds 32
consecutive rows, giving 32 KB load descriptors and, on the store side,
16-row/8 KB-contiguous y runs per (partition, subband) — vs 4 KB at
G=8, worth ~1 us of per-engine packet-overhead.  The Haar 0.5
normalization is folded into a ScalarE pass that also downcasts to bf16
(power-of-2 scale, exact in bf16), so both butterfly stages are plain
adds/subs and stage 1 runs at the DVE's 2x 16-bit rate:
  scale+cast: ACT  xt = 0.5 * it            (f32 -> bf16)
  stage 1 (vertical):   sum/diff of row pairs   (bf16 -> bf16)
  stage 2 (horizontal): even +/- odd columns    (bf16 -> f32 out tile)
One 4 MB f32 store per group: DRAM AP [p:128][s:4, stride 4MB][8 KB run].

SBUF/partition budget (208 KB usable): in 32K*2 + xt 16K*1 + sd 16K*1 +
out 32K*3 = 192 KB.  xt/sd single-buffered is free — all their
producers/consumers are ACT/DVE ops that serialize in program order —
while out*3 (12 MB) absorbs the store backlog that builds up because
engine round-robin is packet-granular and load packets are ~3x larger
(stores drain alone for the last ~50 us; harmless as long as stage 2
never blocks on a free out buffer).

Loads ride the Sync HWDGE ring, stores the Scalar ring.  Ramp trim:
groups 0 and 1 each run load+scale+stage1 in four 1 MB pieces so the
engines spin up sooner; the last group's store is split per stage-2 op
to trim the end-of-kernel drain.

Measured: 169.3-169.7 us across reps (v7/G=8 reference: 171.2).
Engines ~26.6 GB/s each while busy; ~7 us framework preamble and
~2.5 us completion tail are fixed.  Numerics: two bf16 roundings ->
rel err ~4e-3 (gate is 2e-2).
"""

import sys

sys.path.insert(0, "/opt/trn_rl_repo")

import numpy as np

import concourse.bacc as bacc
import concourse.mybir as mybir
from concourse.tile import TileContext

N_CORES = 8
N_PER_CORE = 2  # batches per core
C = 64  # input channels
H = 256
W = 256
G = 16  # channels per group (4 MB loads, 32 rows/partition)
F32 = mybir.dt.float32
BF16 = mybir.dt.bfloat16


def build_nc():
    nc = bacc.Bacc("TRN2", target_bir_lowering=False, debug=False)
    x = nc.dram_tensor("x", [N_PER_CORE, C, H, W], F32, kind="ExternalInput")
    y = nc.dram_tensor("y", [N_PER_CORE, 4 * C, H // 2, W // 2], F32, kind="ExternalOutput")

    n_groups = N_PER_CORE * C // G
    R = H * G // 128 // 2  # row-pairs per partition (16)

    with TileContext(nc) as tc:
        with (
            tc.tile_pool(name="inpool", bufs=2) as inpool,
            tc.tile_pool(name="xtpool", bufs=1) as xtpool,
            tc.tile_pool(name="sdpool", bufs=1) as sdpool,
            tc.tile_pool(name="outpool", bufs=3) as outpool,
        ):
            for gi in range(n_groups):
                n, c0 = divmod(gi * G, C)

                # --- load: pure reshape of the 4 MB contiguous group.
                # it[p, r, t, w] = x[n, c0 + p//8, 32*(p%8) + 2r + t, w]
                it = inpool.tile([128, R * 512], F32, tag="in")
                itv = it[:].rearrange("p (r t w) -> p r t w", r=R, t=2)
                src = x[n, c0 : c0 + G].rearrange(
                    "c (q r t) w -> (c q) r t w", r=R, t=2
                )
                xt = xtpool.tile([128, R * 512], BF16, tag="xt")
                xtv = xt[:].rearrange("p (r t w) -> p r t w", r=R, t=2)
                sd = sdpool.tile([128, R * 512], BF16, tag="sd")
                sdv = sd[:].rearrange("p (v r w) -> p v r w", v=2, r=R)

                npieces = {0: 4, 1: 4}.get(gi, 1)
                step = R // npieces
                pieces = [(i * step, (i + 1) * step) for i in range(npieces)]
                for r0, r1 in pieces:
                    nc.sync.dma_start(out=itv[:, r0:r1], in_=src[:, r0:r1])
                    # --- Haar 0.5 normalization + downcast to bf16 (exact).
                    nc.scalar.mul(xtv[:, r0:r1], itv[:, r0:r1], 0.5)
                    # --- stage 1 (vertical): rows 2t / 2t+1 in a partition.
                    nc.vector.tensor_add(
                        out=sdv[:, 0, r0:r1],
                        in0=xtv[:, r0:r1, 0, :],
                        in1=xtv[:, r0:r1, 1, :],
                    )
                    nc.vector.tensor_sub(
                        out=sdv[:, 1, r0:r1],
                        in0=xtv[:, r0:r1, 0, :],
                        in1=xtv[:, r0:r1, 1, :],
                    )

                # --- stage 2 (horizontal): even/odd column butterfly into
                # one f32 tile holding all four subbands.
                sdj = sd[:].rearrange("p (v r j t) -> p v r j t", v=2, r=R, t=2)
                ot = outpool.tile([128, 4 * R * 128], F32, tag="out")
                ot_v = ot[:].rearrange("p (s r j) -> p s r j", s=4, r=R)
                dst = (
                    y[n]
                    .rearrange("(s c) (q r) j -> s c q (r j)", s=4, r=R)[
                        :, c0 : c0 + G
                    ]
                    .rearrange("s c q f -> (c q) s f")
                )
                ot4 = ot[:].rearrange("p (s f) -> p s f", s=4)
                last = gi == n_groups - 1
                for h, op in ((0, nc.vector.tensor_add), (1, nc.vector.tensor_sub)):
                    op(
                        out=ot_v[:, 2 * h : 2 * h + 2],
                        in0=sdj[..., 0],
                        in1=sdj[..., 1],
                    )
                    if last:
                        # tail trim: stream each subband pair out as soon
                        # as its stage-2 op lands.
                        nc.scalar.dma_start(
                            out=dst[:, 2 * h : 2 * h + 2], in_=ot4[:, 2 * h : 2 * h + 2]
                        )
                if not last:
                    # --- store: one DMA per group.
                    nc.scalar.dma_start(out=dst, in_=ot4)

    nc.finalize()
    return nc


_NC = None


def _get_nc():
    global _NC
    if _NC is None:
        _NC = build_nc()
    return _NC


def kernel(x: np.ndarray) -> np.ndarray:
    from concourse.bass_utils import run_bass_kernel_spmd

    x = np.ascontiguousarray(np.asarray(x), dtype=np.float32)
    assert x.shape == (16, C, H, W), x.shape

    nc = _get_nc()
    in_maps = [
        {"x": x[k * N_PER_CORE : (k + 1) * N_PER_CORE]} for k in range(N_CORES)
    ]
    res = run_bass_kernel_spmd(nc, in_maps, core_ids=list(range(N_CORES)))
    return np.concatenate([r["y"] for r in res.results], axis=0)
